# revision 4
# baseline (speedup 1.0000x reference)
"""Trainium2 Bass kernel for nn_Encoder_30897994727668.

Reference computes (no recurrence, so every timestep independent):
    gates = x @ W_ih.T + b_ih + b_hh            # [B,T,4H], gate order i,f,g,o
    c = sigmoid(i) * tanh(g)                    # f gate unused (c_prev = 0)
    h = sigmoid(o) * tanh(c)
    return (h, c)

Kernel strategy (pure data parallel over B*T across 8 cores), v2:
  * Skip the f gate entirely -> 768 of 1024 gate rows.
  * Fold sigmoid into tanh: sigmoid(z) = (1 + tanh(z/2))/2 by pre-scaling
    the i/o rows of W and b by 0.5 on the host, so ONE ScalarE tanh pass
    (FD=1536 per 2-tile pair) covers all three gates.
  * x is cast f32->fp16 during the DMA load (SWDGE), then transposed
    SBUF->SBUF by the DMA xbar (dma_start_transpose, 128x128 fp16 tiles)
    instead of burning TensorE transpose + VectorE cast cycles.
  * PE does only the gate matmul (fp16, 768 cols/tile) + the bias matmul
    (ones-trick, accumulated into PSUM).
  * tanh(c) is NOT a ScalarE pass: |c| <= 0.85, so a degree-3 odd minimax
    polynomial on the VectorE (2 TT + 1 TS in fp16) is accurate to 2.7e-3
    (end-to-end h rel err ~1e-2 vs the 2e-2 gate).
  * The two 0.5*t+0.5 affines run on GpSimdE (tensor_scalar; GpSimd shares
    only the DVE 2-port pair, and all our DVE TT ops are single-port).
  * h and c are stored as fp16 in DRAM; the host upcasts to f32.
  * Token <-> partition mapping t = macro*2048 + k*128 + p (k-major) keeps
    the per-tile DMA transpose semantics right and DMA runs at 512B.
"""

import sys

if "/opt/trn_rl_repo" not in sys.path:
    sys.path.insert(0, "/opt/trn_rl_repo")

import numpy as np

import concourse.bacc as bacc
import concourse.bass as bass
import concourse.tile as tile
from concourse import mybir
from concourse.bass_utils import run_bass_kernel_spmd
from concourse.tile_rust import add_dep_helper

N_CORES = 8
BATCH = 64
SEQ = 2048
IN = 128          # input features (= K of the matmul = partition count)
H = 256           # hidden
G = 3 * H         # gates kept: i, g, o  (f skipped)
TOKENS = BATCH * SEQ              # 131072
TOK_PER_CORE = TOKENS // N_CORES  # 16384
MACRO_TOK = 2048                  # tokens per macro-iteration
TILES = MACRO_TOK // 128          # 16 token-tiles per macro
MACROS = TOK_PER_CORE // MACRO_TOK  # 8
CH_TILES = 8                      # tiles per chunk (half-macro)
NCHUNKS = MACROS * 2              # 16
PAIRS_PER_CHUNK = CH_TILES // 2   # 4

# tanh(c) ~= c*(K0 + K1*c^2), minimax on [-0.88, 0.88] (maxerr 2.7e-3)
K0 = 0.98370736
K1 = -0.23766349

F32 = mybir.dt.float32
F16 = mybir.dt.float16


def _build_program():
    nc = bacc.Bacc(None, target_bir_lowering=False, debug=False)

    x_d = nc.dram_tensor("x", [TOK_PER_CORE, IN], F32, kind="ExternalInput")
    wt_d = nc.dram_tensor("wt", [IN, G], F16, kind="ExternalInput")
    bias_d = nc.dram_tensor("bias", [G], F16, kind="ExternalInput")
    h_d = nc.dram_tensor("h", [TOK_PER_CORE, H], F16, kind="ExternalOutput")
    c_d = nc.dram_tensor("c", [TOK_PER_CORE, H], F16, kind="ExternalOutput")

    AF = mybir.ActivationFunctionType
    OP = mybir.AluOpType

    with tile.TileContext(nc) as tc:
        with (
            tc.tile_pool(name="consts", bufs=1) as consts,
            tc.tile_pool(name="xin", bufs=2) as xin,
            tc.tile_pool(name="xt", bufs=2) as xtp,
            tc.tile_pool(name="tst", bufs=2) as tstp,
            tc.tile_pool(name="wv", bufs=2) as wvp,
            tc.tile_pool(name="scr", bufs=2) as scr,
            tc.tile_pool(name="outs", bufs=2) as outp,
            tc.tile_pool(name="ps_g", bufs=2, space=bass.MemorySpace.PSUM) as ps_g,
        ):
            ones = consts.tile([128, 128], F16)
            nc.vector.memset(ones, 1.0)
            wt_sb = consts.tile([IN, G], F16)
            nc.sync.dma_start(wt_sb[:], wt_d[:])
            bias_b = consts.tile([128, G], F16)
            nc.vector.memset(bias_b, 0.0)
            nc.sync.dma_start(
                bias_b[0:1, :], bass.AP(bias_d, 0, [[0, 1], [1, G]])
            )

            xT_tiles = [None] * MACROS
            t_tiles = [None] * NCHUNKS

            def load_macro(mac):
                t0 = mac * MACRO_TOK
                x16 = xin.tile([128, TILES, IN], F16, tag="x16", name=f"x16_{mac}")
                xT = xtp.tile([128, TILES, IN], F16, tag="xT", name=f"xT_{mac}")
                x_view = x_d[t0 : t0 + MACRO_TOK, :].rearrange(
                    "(k p) i -> p k i", p=128
                )
                # SWDGE cast load f32 -> fp16; macro 0 in quarters so the
                # first transposes/matmuls start as early as possible
                step = TILES // (4 if mac == 0 else 2)
                for lo in range(0, TILES, step):
                    nc.gpsimd.dma_start(
                        x16[:, lo : lo + step, :], x_view[:, lo : lo + step, :]
                    )
                # xbar transpose per 128x128 tile: [tok, feat] -> [feat, tok]
                for k in range(TILES):
                    nc.sync.dma_start_transpose(xT[:, k, :], x16[:, k, :])
                xT_tiles[mac] = xT

            def emit_pair(q, kp):
                mac, half = divmod(q, 2)
                xT = xT_tiles[mac]
                t_st = t_tiles[q]
                g_ps = ps_g.tile([128, 2, G], F32)  # 3 PSUM banks
                mid_bank_clearer = None
                for j in (0, 1):
                    k = half * CH_TILES + 2 * kp + j
                    # bank-aligned matmul split: tile0 -> 512|256,
                    # tile1 -> 256|512 (pair spans banks b|b+1|b+2)
                    cuts = [(0, 512, True), (512, 768, True)] if j == 0 else [
                        (0, 256, False), (256, 768, True)]
                    for lo, hi, starts in cuts:
                        mm = nc.tensor.matmul(
                            g_ps[:, j, lo:hi], xT[:, k, :], wt_sb[:, lo:hi],
                            start=starts, stop=False, skip_group_check=True,
                        )
                        if j == 0 and lo == 512:
                            # clears has_written for the shared middle bank;
                            # tile1's first mm must come after
                            mid_bank_clearer = mm
                        if j == 1 and lo == 0:
                            add_dep_helper(
                                mm.ins,
                                mid_bank_clearer.ins,
                                reason="shared PSUM bank: overwrite after clear",
                            )
                        nc.tensor.matmul(
                            g_ps[:, j, lo:hi], ones[:], bias_b[:, lo:hi],
                            start=False, stop=True, skip_group_check=True,
                        )
                # one tanh pass over both tiles' [i'|g|o'] (FD=1536)
                nc.scalar.activation(
                    t_st[:, 2 * kp : 2 * kp + 2, :], g_ps[:], AF.Tanh
                )

            def emit_post(q):
                mac, half = divmod(q, 2)
                t_st = t_tiles[q]
                ti = t_st[:, :, 0:H]
                tg = t_st[:, :, H : 2 * H]
                to = t_st[:, :, 2 * H : 3 * H]
                sh = [128, CH_TILES, H]
                # w = sigmoid(i) = 0.5*ti + 0.5   (GpSimd; DVE TTs are 1-port)
                w = wvp.tile(sh, F16, tag="w")
                nc.gpsimd.tensor_scalar(w[:], ti, 0.5, 0.5, OP.mult, OP.add)
                c_st = outp.tile(sh, F16, tag="c", name=f"c{q}")
                nc.vector.tensor_mul(c_st[:], w[:], tg)
                t0 = mac * MACRO_TOK
                sl = slice(half * CH_TILES, (half + 1) * CH_TILES)
                c_view = c_d[t0 : t0 + MACRO_TOK, :].rearrange(
                    "(k p) j -> p k j", p=128
                )
                nc.gpsimd.dma_start(c_view[:, sl, :], c_st[:])
                # u = tanh(c) ~= c*(K0 + K1*c^2)  on VectorE (fp16)
                c2 = scr.tile(sh, F16, tag="c2")
                nc.vector.tensor_mul(c2[:], c_st[:], c_st[:])
                p = scr.tile(sh, F16, tag="p")
                nc.vector.tensor_scalar(p[:], c2[:], K1, K0, OP.mult, OP.add)
                u = scr.tile(sh, F16, tag="u")
                nc.vector.tensor_mul(u[:], p[:], c_st[:])
                # v = sigmoid(o) = 0.5*to + 0.5   (GpSimd)
                v = wvp.tile(sh, F16, tag="v")
                nc.gpsimd.tensor_scalar(v[:], to, 0.5, 0.5, OP.mult, OP.add)
                h_st = outp.tile(sh, F16, tag="h", name=f"h{q}")
                nc.vector.tensor_mul(h_st[:], v[:], u[:])
                h_view = h_d[t0 : t0 + MACRO_TOK, :].rearrange(
                    "(k p) j -> p k j", p=128
                )
                nc.gpsimd.dma_start(h_view[:, sl, :], h_st[:])

            load_macro(0)
            for q in range(NCHUNKS + 1):
                if q < NCHUNKS:
                    mac, half = divmod(q, 2)
                    if half == 0 and mac + 1 < MACROS:
                        load_macro(mac + 1)
                    t_tiles[q] = tstp.tile(
                        [128, CH_TILES, G], F16, tag="t", name=f"t{q}"
                    )
                    for kp in range(PAIRS_PER_CHUNK):
                        emit_pair(q, kp)
                if 1 <= q:
                    emit_post(q - 1)

    nc.compile()
    return nc


_NC_CACHE = None


def _get_nc():
    global _NC_CACHE
    if _NC_CACHE is None:
        _NC_CACHE = _build_program()
    return _NC_CACHE


def _prep_weights(W_ih, b_ih, b_hh):
    W = np.asarray(W_ih, dtype=np.float32)
    b = np.asarray(b_ih, dtype=np.float32) + np.asarray(b_hh, dtype=np.float32)
    Wi, Wg, Wo = W[0:H], W[2 * H : 3 * H], W[3 * H : 4 * H]
    bi, bg, bo = b[0:H], b[2 * H : 3 * H], b[3 * H : 4 * H]
    Wp = np.concatenate([0.5 * Wi, Wg, 0.5 * Wo], axis=0)       # [768, 128]
    bp = np.concatenate([0.5 * bi, bg, 0.5 * bo], axis=0)       # [768]
    wt = np.ascontiguousarray(Wp.T).astype(np.float16)  # [128, 768]
    return wt, np.ascontiguousarray(bp).astype(np.float16)


def kernel(x, W_ih, W_hh, b_ih, b_hh):
    nc = _get_nc()
    x = np.asarray(x, dtype=np.float32).reshape(TOKENS, IN)
    wt, bp = _prep_weights(W_ih, b_ih, b_hh)

    in_maps = []
    for core in range(N_CORES):
        sl = x[core * TOK_PER_CORE : (core + 1) * TOK_PER_CORE]
        in_maps.append({"x": np.ascontiguousarray(sl), "wt": wt, "bias": bp})

    res = run_bass_kernel_spmd(nc, in_maps, core_ids=list(range(N_CORES)))

    h = np.concatenate(
        [np.asarray(res.results[i]["h"], dtype=np.float32) for i in range(N_CORES)],
        axis=0,
    )
    c = np.concatenate(
        [np.asarray(res.results[i]["c"], dtype=np.float32) for i in range(N_CORES)],
        axis=0,
    )
    h = h.reshape(BATCH, SEQ, H)
    c = c.reshape(BATCH, SEQ, H)
    return (h, c)


# revision 7
# speedup vs baseline: 1.9872x; 1.9872x over previous
"""Trainium2 Bass kernel for nn_Encoder_30897994727668.

Reference computes (no recurrence, so every timestep independent):
    gates = x @ W_ih.T + b_ih + b_hh            # [B,T,4H], gate order i,f,g,o
    c = sigmoid(i) * tanh(g)                    # f gate unused (c_prev = 0)
    h = sigmoid(o) * tanh(c)
    return (h, c)

Kernel strategy (pure data parallel over B*T across 8 cores), v4:
  * Skip the f gate entirely -> 768 of 1024 gate rows.
  * Fold sigmoid into tanh: sigmoid(z) = (1 + tanh(z/2))/2 by pre-scaling
    the i/o rows of W and b by 0.5 on the host, so ONE ScalarE tanh pass
    (FD=1536 per 2-tile pair) covers all three gates.
  * x cast f32->fp16 during the DMA load (SWDGE), PE-transposed per
    128x128 tile into a 2-bank PSUM strip, then ONE VectorE cast per
    1024-token chunk brings xT to SBUF fp16 (batched: 8x fewer cast ops
    than per-tile).
  * Gate matmul fp16 (768 cols/tile) + bias matmul (ones-trick), with
    LDWEIGHTS batched per pair: xT0 (2 mms), xT1 (2 mms), ones (4 mms).
  * tanh(c): |c| <= 0.85, degree-3 odd minimax poly on VectorE in fp16
    (maxerr 2.7e-3; end-to-end h rel err ~1e-2 vs the 2e-2 gate).
  * The two 0.5*t+0.5 affines run on GpSimdE (tensor_scalar); all DVE TT
    ops are single-port so they never contend with GpSimd.
  * h and c stored fp16 (HWDGE on SyncE); host upcasts to f32.
  * Token mapping t = chunk*1024 + k*128 + p (k-major).
"""

import sys

if "/opt/trn_rl_repo" not in sys.path:
    sys.path.insert(0, "/opt/trn_rl_repo")

import numpy as np

import concourse.bacc as bacc
import concourse.bass as bass
import concourse.tile as tile
from concourse import mybir
from concourse.bass_utils import run_bass_kernel_spmd
from concourse.masks import make_identity
from concourse.tile_rust import add_dep_helper

N_CORES = 8
BATCH = 64
SEQ = 2048
IN = 128          # input features (= K of the matmul = partition count)
H = 256           # hidden
G = 3 * H         # gates kept: i, g, o  (f skipped)
TOKENS = BATCH * SEQ              # 131072
TOK_PER_CORE = TOKENS // N_CORES  # 16384
CH_TILES = 8                      # 128-token tiles per chunk
CH_TOK = CH_TILES * 128           # 1024
NCHUNKS = TOK_PER_CORE // CH_TOK  # 16
PAIRS_PER_CHUNK = CH_TILES // 2   # 4

# tanh(c) ~= c*(K0 + K1*c^2), minimax on [-0.88, 0.88] (maxerr 2.7e-3)
K0 = 0.98370736
K1 = -0.23766349

F32 = mybir.dt.float32
F16 = mybir.dt.float16


def _build_program():
    nc = bacc.Bacc(None, target_bir_lowering=False, debug=False)

    x_d = nc.dram_tensor("x", [TOK_PER_CORE, IN], F32, kind="ExternalInput")
    wt_d = nc.dram_tensor("wt", [IN, G], F16, kind="ExternalInput")
    bias_d = nc.dram_tensor("bias", [G], F16, kind="ExternalInput")
    h_d = nc.dram_tensor("h", [TOK_PER_CORE, H], F16, kind="ExternalOutput")
    c_d = nc.dram_tensor("c", [TOK_PER_CORE, H], F16, kind="ExternalOutput")

    AF = mybir.ActivationFunctionType
    OP = mybir.AluOpType

    with tile.TileContext(nc) as tc:
        with (
            tc.tile_pool(name="consts", bufs=1) as consts,
            tc.tile_pool(name="xin", bufs=3) as xin,
            tc.tile_pool(name="xt", bufs=2) as xtp,
            tc.tile_pool(name="tst", bufs=2) as tstp,
            tc.tile_pool(name="wv", bufs=2) as wvp,
            tc.tile_pool(name="scr", bufs=2) as scr,
            tc.tile_pool(name="outs", bufs=2) as outp,
            tc.tile_pool(name="ps_g", bufs=2, space=bass.MemorySpace.PSUM) as ps_g,
            tc.tile_pool(name="ps_t", bufs=1, space=bass.MemorySpace.PSUM) as ps_t,
        ):
            ident = consts.tile([128, 128], F16)
            make_identity(nc, ident)
            ones = consts.tile([128, 128], F16)
            nc.vector.memset(ones, 1.0)
            wt_sb = consts.tile([IN, G], F16)
            nc.sync.dma_start(wt_sb[:], wt_d[:])
            bias_b = consts.tile([128, G], F16)
            nc.vector.memset(bias_b, 0.0)
            nc.sync.dma_start(
                bias_b[0:1, :], bass.AP(bias_d, 0, [[0, 1], [1, G]])
            )

            x16_tiles = [None] * NCHUNKS
            xT_tiles = [None] * NCHUNKS
            t_tiles = [None] * NCHUNKS

            def emit_load(q):
                tc0 = q * CH_TOK
                x16 = xin.tile([128, CH_TILES, IN], F16, tag="x16", name=f"x16_{q}")
                x_view = x_d[tc0 : tc0 + CH_TOK, :].rearrange(
                    "(k p) i -> p k i", p=128
                )
                # SWDGE cast load f32 -> fp16; chunk 0 in halves so the first
                # transposes start as early as possible
                step = CH_TILES // (2 if q == 0 else 1)
                for lo in range(0, CH_TILES, step):
                    nc.gpsimd.dma_start(
                        x16[:, lo : lo + step, :], x_view[:, lo : lo + step, :]
                    )
                x16_tiles[q] = x16

            def emit_trans(q):
                x16 = x16_tiles[q]
                xT_ps = ps_t.tile([128, CH_TOK], F16)  # 1 PSUM bank
                for k in range(CH_TILES):
                    nc.tensor.transpose(
                        xT_ps[:, k * 128 : (k + 1) * 128], x16[:, k, :], ident[:]
                    )
                xT = xtp.tile([128, CH_TOK], F16, tag="xT", name=f"xT{q}")
                nc.vector.tensor_copy(xT[:], xT_ps[:])
                xT_tiles[q] = xT

            gate_cuts = {
                0: [(0, 512, True), (512, 768, True)],
                1: [(0, 256, False), (256, 768, True)],
            }

            def emit_pair(q, kp):
                xT = xT_tiles[q]
                t_st = t_tiles[q]
                g_ps = ps_g.tile([128, 2, G], F32)  # 3 PSUM banks
                mid_bank_clearer = None
                for j in (0, 1):
                    k = 2 * kp + j
                    lhs = xT[:, k * 128 : (k + 1) * 128]
                    for lo, hi, starts in gate_cuts[j]:
                        mm = nc.tensor.matmul(
                            g_ps[:, j, lo:hi], lhs, wt_sb[:, lo:hi],
                            start=starts, stop=False, skip_group_check=True,
                        )
                        if j == 0 and lo == 512:
                            # clears has_written for the shared middle bank;
                            # tile1's first mm must come after
                            mid_bank_clearer = mm
                        if j == 1 and lo == 0:
                            add_dep_helper(
                                mm.ins,
                                mid_bank_clearer.ins,
                                reason="shared PSUM bank: overwrite after clear",
                            )
                # bias matmuls last: ones stays stationary across all four
                for j in (0, 1):
                    for lo, hi, _ in gate_cuts[j]:
                        nc.tensor.matmul(
                            g_ps[:, j, lo:hi], ones[:], bias_b[:, lo:hi],
                            start=False, stop=True, skip_group_check=True,
                        )
                # one tanh pass over both tiles' [i'|g|o'] (FD=1536)
                nc.scalar.activation(
                    t_st[:, 2 * kp : 2 * kp + 2, :], g_ps[:], AF.Tanh
                )

            def emit_post(q):
                t_st = t_tiles[q]
                ti = t_st[:, :, 0:H]
                tg = t_st[:, :, H : 2 * H]
                to = t_st[:, :, 2 * H : 3 * H]
                sh = [128, CH_TILES, H]
                tc0 = q * CH_TOK
                # w = sigmoid(i) = 0.5*ti + 0.5   (GpSimd; DVE TTs are 1-port)
                w = wvp.tile(sh, F16, tag="w")
                nc.gpsimd.tensor_scalar(w[:], ti, 0.5, 0.5, OP.mult, OP.add)
                c_st = outp.tile(sh, F16, tag="c", name=f"c{q}")
                nc.vector.tensor_mul(c_st[:], w[:], tg)
                c_view = c_d[tc0 : tc0 + CH_TOK, :].rearrange(
                    "(k p) j -> p k j", p=128
                )
                nc.sync.dma_start(c_view[:], c_st[:])
                # u = tanh(c) ~= c*(K0 + K1*c^2)  on VectorE (fp16)
                c2 = scr.tile(sh, F16, tag="c2")
                nc.vector.tensor_mul(c2[:], c_st[:], c_st[:])
                p = scr.tile(sh, F16, tag="p")
                nc.vector.tensor_scalar(p[:], c2[:], K1, K0, OP.mult, OP.add)
                u = scr.tile(sh, F16, tag="u")
                nc.vector.tensor_mul(u[:], p[:], c_st[:])
                # v = sigmoid(o) = 0.5*to + 0.5   (GpSimd)
                v = wvp.tile(sh, F16, tag="v")
                nc.gpsimd.tensor_scalar(v[:], to, 0.5, 0.5, OP.mult, OP.add)
                h_st = outp.tile(sh, F16, tag="h", name=f"h{q}")
                nc.vector.tensor_mul(h_st[:], v[:], u[:])
                h_view = h_d[tc0 : tc0 + CH_TOK, :].rearrange(
                    "(k p) j -> p k j", p=128
                )
                nc.sync.dma_start(h_view[:], h_st[:])

            emit_load(0)
            emit_load(1)
            emit_trans(0)
            for q in range(NCHUNKS + 1):
                if q < NCHUNKS:
                    if q + 2 < NCHUNKS:
                        emit_load(q + 2)
                    if q + 1 < NCHUNKS:
                        emit_trans(q + 1)
                    t_tiles[q] = tstp.tile(
                        [128, CH_TILES, G], F16, tag="t", name=f"t{q}"
                    )
                    for kp in range(PAIRS_PER_CHUNK):
                        emit_pair(q, kp)
                if 1 <= q:
                    emit_post(q - 1)

    nc.compile()
    return nc


_NC_CACHE = None


def _get_nc():
    global _NC_CACHE
    if _NC_CACHE is None:
        _NC_CACHE = _build_program()
    return _NC_CACHE


def _prep_weights(W_ih, b_ih, b_hh):
    W = np.asarray(W_ih, dtype=np.float32)
    b = np.asarray(b_ih, dtype=np.float32) + np.asarray(b_hh, dtype=np.float32)
    Wi, Wg, Wo = W[0:H], W[2 * H : 3 * H], W[3 * H : 4 * H]
    bi, bg, bo = b[0:H], b[2 * H : 3 * H], b[3 * H : 4 * H]
    Wp = np.concatenate([0.5 * Wi, Wg, 0.5 * Wo], axis=0)       # [768, 128]
    bp = np.concatenate([0.5 * bi, bg, 0.5 * bo], axis=0)       # [768]
    wt = np.ascontiguousarray(Wp.T).astype(np.float16)  # [128, 768]
    return wt, np.ascontiguousarray(bp).astype(np.float16)


def kernel(x, W_ih, W_hh, b_ih, b_hh):
    nc = _get_nc()
    x = np.asarray(x, dtype=np.float32).reshape(TOKENS, IN)
    wt, bp = _prep_weights(W_ih, b_ih, b_hh)

    in_maps = []
    for core in range(N_CORES):
        sl = x[core * TOK_PER_CORE : (core + 1) * TOK_PER_CORE]
        in_maps.append({"x": np.ascontiguousarray(sl), "wt": wt, "bias": bp})

    res = run_bass_kernel_spmd(nc, in_maps, core_ids=list(range(N_CORES)))

    h = np.concatenate(
        [np.asarray(res.results[i]["h"], dtype=np.float32) for i in range(N_CORES)],
        axis=0,
    )
    c = np.concatenate(
        [np.asarray(res.results[i]["c"], dtype=np.float32) for i in range(N_CORES)],
        axis=0,
    )
    h = h.reshape(BATCH, SEQ, H)
    c = c.reshape(BATCH, SEQ, H)
    return (h, c)
